# revision 31
# baseline (speedup 1.0000x reference)
"""Trainium2 Bass kernel for causal Lorentz self-attention.

Problem: B=4, L=4096, D=64 single-head self-attention where
  scores = (2 + 2*<q,k>_L) / scale + bias   (Lorentz inner product)
  causal mask (strict upper triangle) + per-query pad-mask
  attn = softmax(scores);  mu = attn @ v
  out = mu / sqrt(max(|<mu,mu>_L|, eps))

Key algebraic fact used: the softmax denominator cancels in the final
normalization (out = mu_raw / sqrt(|<mu_raw,mu_raw>_L|)), so no row-sum
is computed on device.

Sharding: 2 cores per batch. Each core runs an IDENTICAL static program of
4 "slots" (512 queries each) with static k-extents (8,16,24,32) steps of 128
keys. Which query tiles a slot owns, and where the causal boundary falls, is
encoded purely in host-prepared per-core input data:
  - k iterated DESCENDING from the diagonal, so the 4 boundary steps are
    always steps 0..3 of a slot,
  - slots whose causal extent is shorter than the static extent get
    "poison" K columns (huge negative score -> exp underflows to 0) and
    zero V rows.

Default strategy "k66v2" (~69us HW, vs ~163us for the per-step-DMA "k66"):
  - scores: ONE K=66 bf16 matmul per k-step exploiting Lorentz structure
    (time component as bf16 hi/lo cross-terms, spatial bf16), P fp16 from
    ACT exp, V as an fp16 hi/lo stack on the output dim (free: matmul time
    scales with the 512 streamed queries, not output partitions).
  - all inputs preloaded to SBUF via ~10 large DMAs on ONE trigger queue
    ordered by first use (dma_start triggers serialize ~0.7-1us each on the
    issuing engine; per-step DMAs were 125us of the old baseline).
  - k-steps processed in PAIRS: two score matmuls fill the two banks of a
    [128,1024] PSUM tile, ONE exp covers both (ACT is the steady-state
    bottleneck at ~1us/pair), two AV matmuls consume the halves.
  - one GLOBAL 2-deep skewed pipeline over all (slot,pair) steps plus
    interleaved emission of per-slot post-processing, so the PE never
    idles long enough to drop its HAM p-state (a multi-us gap resets the
    2.4GHz clock to 1.2GHz for ~3us).
  - ACT runs ONLY Exp mid-stream (exp/ln sit in different act table sets;
    each ACT_TABLE_LOAD is 1.3us): the 1/sqrt(|<mu,mu>_L|) normalize is
    batched at the tail as DVE reciprocal + one ACT Sqrt (table prefetched
    by a dummy op right after the last exp) + one broadcast DVE multiply.
  - fp16 output tile (|out|<=~5, adds ~5e-4 rel err) halves the tail DMA.
Strategies kept for experiments: "k66q" (256-query slots, 10% fewer steps
but the 256-row matmul stream cannot hold the PE p-state -> slower),
"k66"/"mixed"/"fp16"/"split"/"f32" (earlier generations).
"""

import os
import numpy as np
import ml_dtypes

import concourse.bass as bass
import concourse.bacc as bacc
import concourse.tile as tile
from concourse import mybir
from concourse import masks as cmasks
from concourse import bass_utils
from concourse._compat import with_exitstack
from contextlib import ExitStack

B, L, D = 4, 4096, 64
EPS = 1e-8
N_CORES = 8
QT = 128                       # queries per q-tile / keys per k-step
SLOT_Q = 512                   # queries per slot (4 q-tiles)
SLOTS = 4                      # slots per core
NQ_CORE = SLOTS * SLOT_Q       # 2048 queries per core
NT_CORE = NQ_CORE // QT        # 16 q-tiles per core
SLOT_EXTENTS = (8, 16, 24, 32)  # static k-steps per slot
TOTAL_STEPS = sum(SLOT_EXTENTS)  # 80
# groups of 4 consecutive q-tiles; group g covers q-tiles 4g..4g+3 and needs
# 4g+4 k-tiles. Half 0 gets groups (0,3,4,7) -> extents (4,16,20,32), half 1
# gets (1,2,5,6) -> (8,12,24,28); both fit elementwise under SLOT_EXTENTS.
HALF_GROUPS = ((0, 3, 4, 7), (1, 2, 5, 6))
# k66q: 8 slots of 256 queries; static extents (4s+4) k-steps cover half 0's
# sorted real extents (2,6,...,30) and equal half 1's (4,8,...,32) exactly.
# 144 k-steps of 256 queries = 72 [512]-equivalents, vs 80 for the 4-slot
# schedule - 10% less exp/matmul work with NO control flow.
SLOT_Q2 = 256
SLOTS2 = 8
EXTS2 = tuple(4 * s + 4 for s in range(SLOTS2))  # (4,8,...,32)
TOTAL_STEPS2 = sum(EXTS2)  # 144
HALF_GROUPS2 = (tuple(range(0, 16, 2)), tuple(range(1, 16, 2)))

_F32 = mybir.dt.float32
_BF16 = mybir.dt.bfloat16
_FP16 = mybir.dt.float16
_BF16_NP = ml_dtypes.bfloat16
# strategy:
#   "k66"   - exploit Lorentz structure: time component (the only large
#             score term) as bf16 hi/lo cross-terms, spatial components as
#             single bf16 -> ONE K=66 score matmul. P fp16 from ACT, V fp16
#             hi/lo stack. 2 MMs/step total.
#   "mixed" - bf16 hi/lo pairs for K/Q, P fp16, V fp16 stack. 3 MMs/step.
#   "fp16"  - fp16 hi/lo pairs for K/Q, P fp16, V fp16 stack. 3 MMs/step,
#             but fp16 matmuls are half-rate on PE.
#   "split" - bf16 hi/lo everywhere incl. P (4 MMs/step + DVE splits)
#   "f32"   - exact fp32 fallback (4x slower matmuls)
_STRATEGY = os.environ.get("KERNEL_MM_DT", "k66v2")
_KSTACK = 66  # rows: [-k0h, -k0l, -k0h, k_space(63)] x [q0h, q0h, q0l, q_space]

_cache = {}


def _ensure_ntff_hook():
    """The agent image lacks ``antenv.axon_hooks``; synthesize it using the
    ctypes NTFF driver from trn_agent_boot so trace=True works."""
    import sys as _sys
    if "antenv.axon_hooks" in _sys.modules:
        return
    try:
        import types as _types
        import antenv  # noqa: F401
        from trn_agent_boot.trn_boot import _ntff_profile_via_ctypes
        hook = _ntff_profile_via_ctypes("/opt/axon/libaxon_pjrt.so")
        m = _types.ModuleType("antenv.axon_hooks")
        m.get_axon_ntff_profile_hook = lambda: hook
        m.set_axon_ntff_profile_hook = lambda h: None
        _sys.modules["antenv.axon_hooks"] = m
    except Exception:
        pass


@with_exitstack
def _body_k66(ctx: ExitStack, tc, aps, bias_val):
    """Single K=66 score matmul per step (Lorentz-structured hi/lo),
    fp16 P/V attention matmul. Skewed pipeline."""
    nc = tc.nc
    PSUM = bass.MemorySpace.PSUM

    const = ctx.enter_context(tc.tile_pool(name="const", bufs=1))
    qdp = ctx.enter_context(tc.tile_pool(name="qdp", bufs=1))
    kdp = ctx.enter_context(tc.tile_pool(name="kdp", bufs=4))
    vnp = ctx.enter_context(tc.tile_pool(name="vnp", bufs=4))
    expp = ctx.enter_context(tc.tile_pool(name="expp", bufs=3))
    stp = ctx.enter_context(tc.tile_pool(name="stp", bufs=3, space=PSUM))
    mup = ctx.enter_context(tc.tile_pool(name="mup", bufs=2, space=PSUM))
    tpp = ctx.enter_context(tc.tile_pool(name="tpp", bufs=2, space=PSUM))
    sbp = ctx.enter_context(tc.tile_pool(name="sbp", bufs=1))
    smallp = ctx.enter_context(tc.tile_pool(name="smallp", bufs=4))
    outp = ctx.enter_context(tc.tile_pool(name="outp", bufs=3))

    ident = const.tile([64, 64], _F32)
    cmasks.make_identity(nc, ident[:])
    bias_t = const.tile([128, 1], _F32)
    nc.vector.memset(bias_t[:], float(bias_val))

    qd_sb = qdp.tile([_KSTACK, NQ_CORE], _BF16)
    for s0 in range(SLOTS):
        c0, c1 = s0 * SLOT_Q, (s0 + 1) * SLOT_Q
        nc.sync.dma_start(qd_sb[:, c0:c1], aps["qd66"][:, c0:c1])

    mu_sb = sbp.tile([64, NQ_CORE], _F32)
    muq_all = sbp.tile([128, NT_CORE * D], _F32)
    ln_all = sbp.tile([128, NT_CORE], _F32)

    step_base = 0
    for s in range(SLOTS):
        ext = SLOT_EXTENTS[s]
        q_lo = s * SLOT_Q
        mu_ps = mup.tile([QT, SLOT_Q], _F32)  # rows 0-63 hi, 64-127 lo

        def st_step(i):
            st = step_base + i
            kt = kdp.tile([_KSTACK, QT], _BF16)
            nc.sync.dma_start(kt[:], aps["kd66"][:, st * QT:(st + 1) * QT])
            ps = stp.tile([QT, SLOT_Q], _F32)
            nc.tensor.matmul(
                ps[:], lhsT=kt[:], rhs=qd_sb[:, q_lo:q_lo + SLOT_Q],
                start=True, stop=True,
            )
            return ps

        def av_step(i, ps):
            st = step_base + i
            vt = vnp.tile([QT, QT], _FP16)
            nc.sync.dma_start(vt[:], aps["vn"][st * QT:(st + 1) * QT, :])
            et = expp.tile([QT, SLOT_Q], _FP16)
            nc.scalar.activation(
                et[:], ps[:], mybir.ActivationFunctionType.Exp,
                bias=bias_t[:], scale=1.0,
            )
            if i < 4:
                nc.gpsimd.affine_select(
                    out=et[:], in_=et[:],
                    compare_op=mybir.AluOpType.is_ge,
                    fill=0.0,
                    base=-QT * (3 - i),
                    pattern=[[1, SLOT_Q]],
                    channel_multiplier=-1,
                )
            nc.tensor.matmul(
                mu_ps[:], lhsT=vt[:], rhs=et[:],
                start=(i == 0), stop=(i == ext - 1),
            )

        ps_prev = st_step(0)
        for i in range(1, ext):
            ps_i = st_step(i)
            av_step(i - 1, ps_prev)
            ps_prev = ps_i
        av_step(ext - 1, ps_prev)
        step_base += ext

        lo_sb = smallp.tile([64, SLOT_Q], _F32, tag="losb")
        nc.scalar.copy(lo_sb[:], mu_ps[64:128, :])
        nc.vector.tensor_add(mu_sb[:, q_lo:q_lo + SLOT_Q], mu_ps[0:64, :], lo_sb[:])

        for q in range(SLOT_Q // QT):
            qt_i = s * (SLOT_Q // QT) + q
            tp = tpp.tile([QT, 64], _F32)
            nc.tensor.transpose(
                tp[:], mu_sb[:, qt_i * QT:(qt_i + 1) * QT], ident[:]
            )
            muq = muq_all[:, qt_i * D:(qt_i + 1) * D]
            nc.scalar.copy(muq, tp[:, :D])
            sq = smallp.tile([QT, D], _F32)
            nc.vector.tensor_mul(sq[:], muq, muq)
            red = smallp.tile([QT, 1], _F32)
            nc.vector.reduce_sum(red[:], sq[:], axis=mybir.AxisListType.X)
            nc.vector.scalar_tensor_tensor(
                out=ln_all[:, qt_i:qt_i + 1],
                in0=sq[:, 0:1],
                scalar=2.0,
                in1=red[:],
                op0=mybir.AluOpType.mult,
                op1=mybir.AluOpType.subtract,
            )

        # per-slot normalize: 1/sqrt(x) = exp(-0.5*ln(x)); Ln and Exp share
        # one ACT table set, so no table switch and no end-of-kernel phase.
        lns = ln_all[:, s * 4:(s + 1) * 4]
        lnt = smallp.tile([128, 4], _F32, tag="lnt")
        nc.scalar.activation(lnt[:], lns, mybir.ActivationFunctionType.Ln)
        invs = smallp.tile([128, 4], _F32, tag="invs")
        nc.scalar.activation(
            invs[:], lnt[:], mybir.ActivationFunctionType.Exp,
            bias=0.0, scale=-0.5,
        )
        for q in range(SLOT_Q // QT):
            qt_i = s * (SLOT_Q // QT) + q
            ot = outp.tile([QT, D], _F32)
            nc.vector.tensor_scalar_mul(
                ot[:], muq_all[:, qt_i * D:(qt_i + 1) * D], invs[:, q:q + 1]
            )
            nc.sync.dma_start(aps["out"][qt_i * QT:(qt_i + 1) * QT, :], ot[:])


@with_exitstack
def _body_k66v2(ctx: ExitStack, tc, aps, bias_val):
    """k66 score matmul + fp16 hi/lo V, restructured for engine occupancy:

    - ALL K/V/Q data preloaded into SBUF via a handful of large DMAs
      (per-step dma_start triggers serialize on the issuing sequencer at
      ~600-700ns each; the baseline spent 125us there).
    - k-steps processed in PAIRS: two score matmuls write the two bank
      halves of one [128,1024] PSUM tile, ONE exp activation covers both
      (halves ACT per-op overhead), two AV matmuls consume the halves.
    - causal boundary masks applied as a DVE fp16 multiply (bmask), not a
      gpsimd affine_select on the exp->AV critical path.
    - no scalar.copy anywhere: ACT runs only Exp/Ln (one table set, no
      ACT_TABLE_LOAD churn); PSUM->SBUF moves go to DVE, SBUF-only
      elementwise post-processing goes to Pool (gpsimd).
    - PE warm-up matmuls during the initial DMA window (HAM p-state ramp).
    """
    nc = tc.nc
    PSUM = bass.MemorySpace.PSUM

    const = ctx.enter_context(tc.tile_pool(name="const", bufs=1))
    datap = ctx.enter_context(tc.tile_pool(name="datap", bufs=1))
    expp = ctx.enter_context(tc.tile_pool(name="expp", bufs=4))
    stp = ctx.enter_context(tc.tile_pool(name="stp", bufs=2, space=PSUM))
    mup = ctx.enter_context(tc.tile_pool(name="mup", bufs=2, space=PSUM))
    tpp = ctx.enter_context(tc.tile_pool(name="tpp", bufs=2, space=PSUM))
    smallp = ctx.enter_context(tc.tile_pool(name="smallp", bufs=4))

    # warm-up source memset FIRST so the PE can start ramping ASAP
    wsrc = const.tile([_KSTACK, SLOT_Q], _BF16)
    nc.vector.memset(wsrc[:], 0.0)
    bias_t = const.tile([128, 1], _F32)
    nc.vector.memset(bias_t[:], float(bias_val))

    # ---- input preloads. ALL triggers go on ONE queue (sync) ordered by
    # first use: qd+kd0 gate the first score matmul, then each slot's vn
    # before the next slot's kd. (Splitting across two trigger engines put
    # the 2.6MB vn stream on a queue that monopolized all 16 DMA engines
    # while kd0/qd crawled -> first matmul waited ~8us extra.)
    qd_sb = datap.tile([_KSTACK, NQ_CORE], _BF16)
    kd_sb = {}
    vn_sb = {}
    base = 0
    for s0 in range(SLOTS):
        ext = SLOT_EXTENTS[s0]
        c0 = s0 * SLOT_Q
        nc.sync.dma_start(qd_sb[:, c0:c0 + SLOT_Q], aps["qd66"][:, c0:c0 + SLOT_Q])
        kd_sb[s0] = datap.tile([_KSTACK, ext * QT], _BF16, tag=f"kd{s0}",
                               name=f"kd_sb{s0}")
        if s0 == 0:
            # split slot 0's K so the very first score matmul waits on the
            # minimum number of bytes (qd chunk + 2 k-tiles)
            nc.sync.dma_start(kd_sb[0][:, 0:2 * QT],
                              aps["kd66"][:, base * QT:(base + 2) * QT])
            nc.sync.dma_start(kd_sb[0][:, 2 * QT:ext * QT],
                              aps["kd66"][:, (base + 2) * QT:(base + ext) * QT])
        else:
            nc.sync.dma_start(kd_sb[s0][:], aps["kd66"][:, base * QT:(base + ext) * QT])
        vn_sb[s0] = datap.tile([128, ext * QT], _FP16, tag=f"vn{s0}",
                               name=f"vn_sb{s0}")
        nc.sync.dma_start(vn_sb[s0][:], aps["vnt"][:, base * QT:(base + ext) * QT])
        base += ext

    # ---- causal boundary masks: step i<4 keeps (k,q) iff q - k - 128*(3-i) >= 0
    bmask = const.tile([QT, 4 * SLOT_Q], _FP16)
    nc.gpsimd.memset(bmask[:], 1.0)
    for i in range(4):
        nc.gpsimd.affine_select(
            out=bmask[:, i * SLOT_Q:(i + 1) * SLOT_Q],
            in_=bmask[:, i * SLOT_Q:(i + 1) * SLOT_Q],
            compare_op=mybir.AluOpType.is_ge,
            fill=0.0,
            base=-QT * (3 - i),
            pattern=[[1, SLOT_Q]],
            channel_multiplier=-1,
        )
    ident = const.tile([64, 64], _F32)
    cmasks.make_identity(nc, ident[:])

    # ---- PE warm-up bridging the DMA window: enough matmuls that the PE
    # is never idle before the first real score matmul's data lands (a gap
    # resets the HAM p-state ramp), but not so many that the queue delays
    # the real stream (16+ pushed it out by ~6us)
    for w in range(5):
        wps = tpp.tile([QT, SLOT_Q], _F32, tag="tp")
        nc.tensor.matmul(wps[:], lhsT=wsrc[:, 0:QT], rhs=wsrc[:],
                         start=True, stop=True)

    mu_sb = datap.tile([64, NQ_CORE], _F32)
    muq_all = datap.tile([128, NT_CORE * D], _F32)
    ln_all = datap.tile([128, NT_CORE], _F32)
    out_sb = datap.tile([128, NT_CORE * D], _FP16)

    # Post-processing of slot s is EMITTED interleaved into slot s+1's pair
    # loop so the PE never sees a multi-us gap at slot boundaries (a gap
    # both stalls PE and resets the HAM p-state ramp back to 1.2 GHz).
    post_q = []

    def drain_post():
        if post_q:
            post_q.pop(0)()

    def make_merge(s, mu_ps, q_lo, tail):
        def f():
            # mu = hi half + lo half (PSUM reads: DVE/ACT only). At the
            # kernel tail ACT is idle, so give it the copy (Copy needs no
            # table load - it lives in every act table set).
            lo_sb = smallp.tile([64, SLOT_Q], _F32, tag="losb", name="lo_sb")
            if tail:
                nc.scalar.copy(lo_sb[:], mu_ps[64:128, :])
            else:
                nc.vector.tensor_copy(lo_sb[:], mu_ps[64:128, :])
            nc.vector.tensor_add(mu_sb[:, q_lo:q_lo + SLOT_Q],
                                 mu_ps[0:64, :], lo_sb[:])
        return f

    def make_chain(s, q, tail):
        def f():
            qt_i = s * (SLOT_Q // QT) + q
            tp = tpp.tile([QT, 64], _F32, tag="tp", name="tp")
            nc.tensor.transpose(
                tp[:], mu_sb[:, qt_i * QT:(qt_i + 1) * QT], ident[:]
            )
            muq = muq_all[:, qt_i * D:(qt_i + 1) * D]
            nc.vector.tensor_copy(muq, tp[:, :D])
            sq = smallp.tile([QT, D], _F32, name="sq")
            red = smallp.tile([QT, 1], _F32, name="red")
            if tail:
                # ACT square with free-dim accumulator: sq + row-sum in one op
                nc.scalar.activation(
                    sq[:], muq, mybir.ActivationFunctionType.Square,
                    accum_out=red[:],
                )
            else:
                nc.vector.tensor_mul(sq[:], muq, muq)
                nc.vector.reduce_sum(red[:], sq[:], axis=mybir.AxisListType.X)
            # |l| = -l = 2*mu0^2 - sum(mu_d^2)  (l is always < 0 here)
            nc.vector.scalar_tensor_tensor(
                out=ln_all[:, qt_i:qt_i + 1],
                in0=sq[:, 0:1],
                scalar=2.0,
                in1=red[:],
                op0=mybir.AluOpType.mult,
                op1=mybir.AluOpType.subtract,
            )
        return f

    mu_ps_map = {}

    def get_mu(s):
        if s not in mu_ps_map:
            mu_ps_map[s] = mup.tile([QT, SLOT_Q], _F32, name="mu_ps")
        return mu_ps_map[s]

    def st_pair(s, j):
        q_lo = s * SLOT_Q
        ps = stp.tile([QT, 2 * SLOT_Q], _F32, name="ps")  # 2 PSUM banks
        for h in range(2):
            i = 2 * j + h
            nc.tensor.matmul(
                ps[:, h * SLOT_Q:(h + 1) * SLOT_Q],
                lhsT=kd_sb[s][:, i * QT:(i + 1) * QT],
                rhs=qd_sb[:, q_lo:q_lo + SLOT_Q],
                start=True, stop=True,
            )
        return ps

    def av_pair(s, j, ps):
        ext = SLOT_EXTENTS[s]
        mu_ps = get_mu(s)
        et = expp.tile([QT, 2 * SLOT_Q], _FP16, name="et")
        if j == 0:
            # pair 0 = the two deepest diagonal steps: cols 0:384 (step 0,
            # q < 384) and 512:768 (step 1, q < 256) are causally dead for
            # EVERY k row - zero them on Pool and exp only the live ranges
            nc.gpsimd.memset(et[:, 0:384], 0.0)
            nc.gpsimd.memset(et[:, SLOT_Q:SLOT_Q + 256], 0.0)
            nc.scalar.activation(
                et[:, 384:SLOT_Q], ps[:, 384:SLOT_Q],
                mybir.ActivationFunctionType.Exp, bias=bias_t[:], scale=1.0,
            )
            nc.scalar.activation(
                et[:, SLOT_Q + 256:2 * SLOT_Q], ps[:, SLOT_Q + 256:2 * SLOT_Q],
                mybir.ActivationFunctionType.Exp, bias=bias_t[:], scale=1.0,
            )
            nc.vector.tensor_mul(
                et[:, 384:SLOT_Q], et[:, 384:SLOT_Q], bmask[:, 384:SLOT_Q],
            )
            nc.vector.tensor_mul(
                et[:, SLOT_Q + 256:2 * SLOT_Q], et[:, SLOT_Q + 256:2 * SLOT_Q],
                bmask[:, SLOT_Q + 256:2 * SLOT_Q],
            )
        else:
            nc.scalar.activation(
                et[:], ps[:], mybir.ActivationFunctionType.Exp,
                bias=bias_t[:], scale=1.0,
            )
        if j == 1:
            # steps 2,3 still straddle the causal boundary; one fused
            # [128,1024] fp16 multiply zeroes the upper-triangle part
            nc.vector.tensor_mul(
                et[:], et[:],
                bmask[:, 2 * SLOT_Q:4 * SLOT_Q],
            )
        for h in range(2):
            i = 2 * j + h
            nc.tensor.matmul(
                mu_ps[:],
                lhsT=vn_sb[s][:, i * QT:(i + 1) * QT],
                rhs=et[:, h * SLOT_Q:(h + 1) * SLOT_Q],
                start=(i == 0), stop=(i == ext - 1),
            )
        if i == ext - 1:
            tail = s == SLOTS - 1
            post_q.append(make_merge(s, mu_ps, s * SLOT_Q, tail))
            for q in range(SLOT_Q // QT):
                post_q.append(make_chain(s, q, tail))

    # single GLOBAL 2-deep skewed pipeline over all (slot, pair) steps:
    # slot s+1's score matmuls keep the PE dense while slot s's last exps
    # drain, so the PE never idles long enough to lose its p-state.
    sched = [(s, j) for s in range(SLOTS) for j in range(SLOT_EXTENTS[s] // 2)]
    pend = []
    for (s, j) in sched:
        pend.append((s, j, st_pair(s, j)))
        if len(pend) > 2:
            s0, j0, ps0 = pend.pop(0)
            av_pair(s0, j0, ps0)
            drain_post()
    while pend:
        s0, j0, ps0 = pend.pop(0)
        av_pair(s0, j0, ps0)
        if len(pend) == 1:
            # prefetch the Sqrt act table right after the LAST exp: the
            # load overlaps the tail post-processing on DVE, and every
            # other tail ACT op (copy/square) lives in the sqrt set too.
            dum = smallp.tile([128, 1], _F32, name="dum")
            nc.scalar.activation(dum[:], bias_t[:],
                                 mybir.ActivationFunctionType.Sqrt)
        if pend:
            # no score matmuls remain to overlap the final exp waits; keep
            # the PE busy with a dummy so the HAM p-state survives into the
            # tail transposes
            wps = tpp.tile([QT, SLOT_Q], _F32, tag="tp")
            nc.tensor.matmul(wps[:], lhsT=wsrc[:, 0:QT], rhs=wsrc[:],
                             start=True, stop=True)
        drain_post()
    while post_q:
        drain_post()

    # batched final normalize: 1/sqrt via DVE reciprocal + ONE ACT Sqrt
    # (single table load, at the tail where ACT is otherwise idle)
    lnr = smallp.tile([128, NT_CORE], _F32, name="lnr")
    nc.vector.reciprocal(lnr[:], ln_all[:])
    invs = smallp.tile([128, NT_CORE], _F32, name="invs")
    nc.scalar.activation(invs[:], lnr[:], mybir.ActivationFunctionType.Sqrt)
    # broadcast multiply + output DMA in two halves so the first DMA
    # overlaps the second half's multiply
    HT = NT_CORE // 2
    for hh in range(2):
        cs = slice(hh * HT * D, (hh + 1) * HT * D)
        nc.vector.tensor_mul(
            out_sb[:, cs].rearrange("p (t d) -> p t d", d=D),
            muq_all[:, cs].rearrange("p (t d) -> p t d", d=D),
            invs[:, hh * HT:(hh + 1) * HT].unsqueeze(2).broadcast_to(
                [128, HT, D]),
        )
        nc.sync.dma_start(aps["out"][:, cs], out_sb[:, cs])


@with_exitstack
def _body_k66q(ctx: ExitStack, tc, aps, bias_val):
    """k66v2 with 8 slots of 256 queries and QUAD-fused exp: four [128,256]
    score matmuls fill one [128,1024] PSUM tile, one exp covers all four.
    Static k-extents (4,8,...,32) match half 1 exactly and cover half 0
    sorted, so only 144 k-steps (= 72 [512]-equivalents) run vs 80."""
    nc = tc.nc
    PSUM = bass.MemorySpace.PSUM

    const = ctx.enter_context(tc.tile_pool(name="const", bufs=1))
    datap = ctx.enter_context(tc.tile_pool(name="datap", bufs=1))
    expp = ctx.enter_context(tc.tile_pool(name="expp", bufs=3))
    stp = ctx.enter_context(tc.tile_pool(name="stp", bufs=2, space=PSUM))
    mup = ctx.enter_context(tc.tile_pool(name="mup", bufs=2, space=PSUM))
    tpp = ctx.enter_context(tc.tile_pool(name="tpp", bufs=2, space=PSUM))
    smallp = ctx.enter_context(tc.tile_pool(name="smallp", bufs=4))

    wsrc = const.tile([_KSTACK, SLOT_Q], _BF16)
    nc.vector.memset(wsrc[:], 0.0)
    bias_t = const.tile([128, 1], _F32)
    nc.vector.memset(bias_t[:], float(bias_val))

    qd_sb = datap.tile([_KSTACK, NQ_CORE], _BF16)
    nc.sync.dma_start(qd_sb[:], aps["qd66"][:])
    # group slot chunks so the trigger count stays small (each dma_start
    # serializes ~900ns on the sync queue): slots {0-3}, {4,5}, {6}, {7}
    kd_sb = {}
    vn_sb = {}
    slot_of = {}
    groups_dma = ((0,), (1,), (2, 3), (4, 5), (6,), (7,))
    base = 0
    for gi, grp in enumerate(groups_dma):
        ext_g = sum(EXTS2[s] for s in grp)
        kd_g = datap.tile([_KSTACK, ext_g * QT], _BF16, tag=f"kdg{gi}",
                          name=f"kd_g{gi}")
        nc.sync.dma_start(kd_g[:], aps["kd66"][:, base * QT:(base + ext_g) * QT])
        vn_g = datap.tile([128, ext_g * QT], _FP16, tag=f"vng{gi}",
                          name=f"vn_g{gi}")
        nc.sync.dma_start(vn_g[:], aps["vnt"][:, base * QT:(base + ext_g) * QT])
        off = 0
        for s in grp:
            kd_sb[s] = kd_g[:, off * QT:(off + EXTS2[s]) * QT]
            vn_sb[s] = vn_g[:, off * QT:(off + EXTS2[s]) * QT]
            off += EXTS2[s]
        base += ext_g

    # boundary masks for steps 0,1 (the diagonal pair of each slot):
    # step i keeps (k,q) iff q - k - 128*(1-i) >= 0, q in [0,256)
    bmask = const.tile([QT, 2 * SLOT_Q2], _FP16)
    nc.gpsimd.memset(bmask[:], 1.0)
    for i in range(2):
        nc.gpsimd.affine_select(
            out=bmask[:, i * SLOT_Q2:(i + 1) * SLOT_Q2],
            in_=bmask[:, i * SLOT_Q2:(i + 1) * SLOT_Q2],
            compare_op=mybir.AluOpType.is_ge,
            fill=0.0,
            base=-QT * (1 - i),
            pattern=[[1, SLOT_Q2]],
            channel_multiplier=-1,
        )
    ident = const.tile([64, 64], _F32)
    cmasks.make_identity(nc, ident[:])

    for w in range(5):
        wps = tpp.tile([QT, SLOT_Q], _F32, tag="tp")
        nc.tensor.matmul(wps[:], lhsT=wsrc[:, 0:QT], rhs=wsrc[:],
                         start=True, stop=True)

    mu_sb = datap.tile([64, NQ_CORE], _F32)
    muq_all = datap.tile([128, NT_CORE * D], _F32)
    ln_all = datap.tile([128, NT_CORE], _F32)
    out_sb = datap.tile([128, NT_CORE * D], _F32)

    post_q = []

    def drain_post():
        if post_q:
            post_q.pop(0)()

    def make_merge(s, mu_ps, q_lo, tail):
        def f():
            lo_sb = smallp.tile([64, SLOT_Q2], _F32, tag="losb", name="lo_sb")
            if tail:
                nc.scalar.copy(lo_sb[:], mu_ps[64:128, :])
            else:
                nc.vector.tensor_copy(lo_sb[:], mu_ps[64:128, :])
            nc.vector.tensor_add(mu_sb[:, q_lo:q_lo + SLOT_Q2],
                                 mu_ps[0:64, :], lo_sb[:])
        return f

    def make_chain(s, q, tail):
        def f():
            qt_i = s * (SLOT_Q2 // QT) + q
            tp = tpp.tile([QT, 64], _F32, tag="tp", name="tp")
            nc.tensor.transpose(
                tp[:], mu_sb[:, qt_i * QT:(qt_i + 1) * QT], ident[:]
            )
            muq = muq_all[:, qt_i * D:(qt_i + 1) * D]
            nc.vector.tensor_copy(muq, tp[:, :D])
            sq = smallp.tile([QT, D], _F32, name="sq")
            red = smallp.tile([QT, 1], _F32, name="red")
            if tail:
                nc.scalar.activation(
                    sq[:], muq, mybir.ActivationFunctionType.Square,
                    accum_out=red[:],
                )
            else:
                nc.vector.tensor_mul(sq[:], muq, muq)
                nc.vector.reduce_sum(red[:], sq[:], axis=mybir.AxisListType.X)
            nc.vector.scalar_tensor_tensor(
                out=ln_all[:, qt_i:qt_i + 1],
                in0=sq[:, 0:1],
                scalar=2.0,
                in1=red[:],
                op0=mybir.AluOpType.mult,
                op1=mybir.AluOpType.subtract,
            )
        return f

    mu_ps_map = {}

    def get_mu(s):
        if s not in mu_ps_map:
            mu_ps_map[s] = mup.tile([QT, SLOT_Q2], _F32, name="mu_ps")
        return mu_ps_map[s]

    def st_quad(s, u):
        q_lo = s * SLOT_Q2
        ps = stp.tile([QT, 4 * SLOT_Q2], _F32, name="ps")  # 2 PSUM banks
        for h in range(4):
            i = 4 * u + h
            nc.tensor.matmul(
                ps[:, h * SLOT_Q2:(h + 1) * SLOT_Q2],
                lhsT=kd_sb[s][:, i * QT:(i + 1) * QT],
                rhs=qd_sb[:, q_lo:q_lo + SLOT_Q2],
                start=True, stop=True,
            )  # kd_sb[s] is an AP view into its DMA group tile
        return ps

    def av_quad(s, u, ps):
        ext = EXTS2[s]
        mu_ps = get_mu(s)
        et = expp.tile([QT, 4 * SLOT_Q2], _FP16, name="et")
        nc.scalar.activation(
            et[:], ps[:], mybir.ActivationFunctionType.Exp,
            bias=bias_t[:], scale=1.0,
        )
        if u == 0:
            # steps 0,1 are the diagonal pair: mask the first et half
            nc.vector.tensor_mul(
                et[:, 0:2 * SLOT_Q2], et[:, 0:2 * SLOT_Q2], bmask[:],
            )
        for h in range(4):
            i = 4 * u + h
            nc.tensor.matmul(
                mu_ps[:],
                lhsT=vn_sb[s][:, i * QT:(i + 1) * QT],
                rhs=et[:, h * SLOT_Q2:(h + 1) * SLOT_Q2],
                start=(i == 0), stop=(i == ext - 1),
            )
        if i == ext - 1:
            tail = s == SLOTS2 - 1
            post_q.append(make_merge(s, mu_ps, s * SLOT_Q2, tail))
            for q in range(SLOT_Q2 // QT):
                post_q.append(make_chain(s, q, tail))

    sched = [(s, u) for s in range(SLOTS2) for u in range(EXTS2[s] // 4)]
    pend = []
    for (s, u) in sched:
        pend.append((s, u, st_quad(s, u)))
        if len(pend) > 2:
            s0, u0, ps0 = pend.pop(0)
            av_quad(s0, u0, ps0)
            drain_post()
    while pend:
        s0, u0, ps0 = pend.pop(0)
        av_quad(s0, u0, ps0)
        if len(pend) == 1:
            dum = smallp.tile([128, 1], _F32, name="dum")
            nc.scalar.activation(dum[:], bias_t[:],
                                 mybir.ActivationFunctionType.Sqrt)
        drain_post()
    while post_q:
        drain_post()

    lnr = smallp.tile([128, NT_CORE], _F32, name="lnr")
    nc.vector.reciprocal(lnr[:], ln_all[:])
    invs = smallp.tile([128, NT_CORE], _F32, name="invs")
    nc.scalar.activation(invs[:], lnr[:], mybir.ActivationFunctionType.Sqrt)
    nc.vector.tensor_mul(
        out_sb[:].rearrange("p (t d) -> p t d", d=D),
        muq_all[:].rearrange("p (t d) -> p t d", d=D),
        invs[:].unsqueeze(2).broadcast_to([128, NT_CORE, D]),
    )
    nc.sync.dma_start(aps["out"][:], out_sb[:])


@with_exitstack
def _body_fp16(ctx: ExitStack, tc, aps, bias_val, kq_dt=_FP16):
    """hi/lo-pair strategy with software-pipelined (skewed) step loop and
    per-slot preloaded K/V (per-step DMA triggers serialize on the sync
    sequencer at ~590ns each, so they must be batched).
    kq_dt: dtype of the K/Q score operands (bf16 = PE full rate)."""
    nc = tc.nc
    PSUM = bass.MemorySpace.PSUM

    const = ctx.enter_context(tc.tile_pool(name="const", bufs=1))
    qdp = ctx.enter_context(tc.tile_pool(name="qdp", bufs=1))
    kdp = ctx.enter_context(tc.tile_pool(name="kdp", bufs=1))
    vnp = ctx.enter_context(tc.tile_pool(name="vnp", bufs=1))
    expp = ctx.enter_context(tc.tile_pool(name="expp", bufs=3))
    stp = ctx.enter_context(tc.tile_pool(name="stp", bufs=3, space=PSUM))
    mup = ctx.enter_context(tc.tile_pool(name="mup", bufs=2, space=PSUM))
    tpp = ctx.enter_context(tc.tile_pool(name="tpp", bufs=2, space=PSUM))
    sbp = ctx.enter_context(tc.tile_pool(name="sbp", bufs=1))
    smallp = ctx.enter_context(tc.tile_pool(name="smallp", bufs=4))
    outp = ctx.enter_context(tc.tile_pool(name="outp", bufs=3))

    ident = const.tile([64, 64], _F32)
    cmasks.make_identity(nc, ident[:])
    bias_t = const.tile([128, 1], _F32)
    nc.vector.memset(bias_t[:], float(bias_val))

    # PE warm-up: ~16 dummy matmuls during the initial DMA window so the
    # HAM clock-gate reaches 2.4 GHz before the first real matmul.
    wsrc = const.tile([QT, SLOT_Q], kq_dt)
    nc.gpsimd.memset(wsrc[:], 0.0)
    for w in range(16):
        wps = tpp.tile([QT, SLOT_Q], _F32, tag="warm", bufs=1)
        nc.tensor.matmul(wps[:], lhsT=wsrc[:, 0:QT], rhs=wsrc[:],
                         start=True, stop=True)

    # causal boundary masks (fp16 ones/zeros incl. diagonal triangle),
    # applied with a DVE multiply instead of a gpsimd affine_select on the
    # exp->AV critical path.
    bmask = const.tile([QT, 4, SLOT_Q], _FP16)
    nc.vector.memset(bmask[:], 1.0)
    for i in range(4):
        nc.gpsimd.affine_select(
            out=bmask[:, i, :], in_=bmask[:, i, :],
            compare_op=mybir.AluOpType.is_ge,
            fill=0.0,
            base=-QT * (3 - i),
            pattern=[[1, SLOT_Q]],
            channel_multiplier=-1,
        )

    qdh_sb = qdp.tile([128, NQ_CORE], kq_dt)
    qdl_sb = qdp.tile([64, NQ_CORE], kq_dt)
    kd_sb = {}
    vn_sb = {}
    base = 0
    for s0 in range(SLOTS):
        ext = SLOT_EXTENTS[s0]
        c0, c1 = s0 * SLOT_Q, (s0 + 1) * SLOT_Q
        nc.sync.dma_start(qdh_sb[:, c0:c1], aps["qdh"][:, c0:c1])
        nc.sync.dma_start(qdl_sb[:, c0:c1], aps["qdl"][:, c0:c1])
        kd_sb[s0] = kdp.tile([QT, ext * QT], kq_dt, tag=f"kd{s0}", name=f"kd_sb{s0}")
        nc.sync.dma_start(kd_sb[s0][:], aps["kd"][:, base * QT:(base + ext) * QT])
        vn_sb[s0] = vnp.tile([QT, ext, QT], _FP16, tag=f"vn{s0}", name=f"vn_sb{s0}")
        vsrc = aps["vn"][base * QT:(base + ext) * QT, :].rearrange(
            "(t p) c -> p t c", p=QT)
        nc.sync.dma_start(vn_sb[s0][:], vsrc)
        base += ext

    mu_sb = sbp.tile([64, NQ_CORE], _F32)
    muq_all = sbp.tile([128, NT_CORE * D], _F32)
    ln_all = sbp.tile([128, NT_CORE], _F32)

    step_base = 0
    for s in range(SLOTS):
        ext = SLOT_EXTENTS[s]
        q_lo = s * SLOT_Q
        mu_ps = mup.tile([QT, SLOT_Q], _F32)  # rows 0-63 hi, 64-127 lo

        def st_step(i):
            kt = kd_sb[s][:, i * QT:(i + 1) * QT]
            ps = stp.tile([QT, SLOT_Q], _F32)
            nc.tensor.matmul(
                ps[:], lhsT=kt, rhs=qdh_sb[:, q_lo:q_lo + SLOT_Q],
                start=True, stop=False,
            )
            nc.tensor.matmul(
                ps[:], lhsT=kt[0:64, :], rhs=qdl_sb[:, q_lo:q_lo + SLOT_Q],
                start=False, stop=True,
            )
            return ps

        def av_step(i, ps):
            vt = vn_sb[s][:, i, :]
            et = expp.tile([QT, SLOT_Q], _FP16)
            nc.scalar.activation(
                et[:], ps[:], mybir.ActivationFunctionType.Exp,
                bias=bias_t[:], scale=1.0,
            )
            if i < 4:
                # step i's k-tile is the (3-i)'th q-tile block's diagonal
                nc.vector.tensor_mul(et[:], et[:], bmask[:, i, :])
            nc.tensor.matmul(
                mu_ps[:], lhsT=vt, rhs=et[:],
                start=(i == 0), stop=(i == ext - 1),
            )

        # 2-deep skewed pipeline: S_T(i+2) runs on PE before AV(i), covering
        # the exp latency (and the boundary-mask multiply) on ACT/DVE.
        pending = [st_step(0), st_step(1)]
        for i in range(2, ext):
            pending.append(st_step(i))
            av_step(i - 2, pending.pop(0))
        av_step(ext - 2, pending.pop(0))
        av_step(ext - 1, pending.pop(0))
        step_base += ext

        # mu = hi half + lo half (one PSUM operand max per DVE op)
        lo_sb = smallp.tile([64, SLOT_Q], _F32, tag="losb")
        nc.scalar.copy(lo_sb[:], mu_ps[64:128, :])
        nc.vector.tensor_add(mu_sb[:, q_lo:q_lo + SLOT_Q], mu_ps[0:64, :], lo_sb[:])

        for q in range(SLOT_Q // QT):
            qt_i = s * (SLOT_Q // QT) + q
            tp = tpp.tile([QT, 64], _F32)
            nc.tensor.transpose(
                tp[:], mu_sb[:, qt_i * QT:(qt_i + 1) * QT], ident[:]
            )
            muq = muq_all[:, qt_i * D:(qt_i + 1) * D]
            nc.scalar.copy(muq, tp[:, :D])
            sq = smallp.tile([QT, D], _F32)
            nc.vector.tensor_mul(sq[:], muq, muq)
            red = smallp.tile([QT, 1], _F32)
            nc.vector.reduce_sum(red[:], sq[:], axis=mybir.AxisListType.X)
            # |l| = -l = 2*mu0^2 - sum(mu_d^2)  (l is always < 0 here)
            nc.vector.scalar_tensor_tensor(
                out=ln_all[:, qt_i:qt_i + 1],
                in0=sq[:, 0:1],
                scalar=2.0,
                in1=red[:],
                op0=mybir.AluOpType.mult,
                op1=mybir.AluOpType.subtract,
            )

        # per-slot normalize: 1/sqrt(x) = exp(-0.5*ln(x)); Ln and Exp share
        # one ACT table set, so no table switch and no end-of-kernel phase.
        lns = ln_all[:, s * 4:(s + 1) * 4]
        lnt = smallp.tile([128, 4], _F32, tag="lnt")
        nc.scalar.activation(lnt[:], lns, mybir.ActivationFunctionType.Ln)
        invs = smallp.tile([128, 4], _F32, tag="invs")
        nc.scalar.activation(
            invs[:], lnt[:], mybir.ActivationFunctionType.Exp,
            bias=0.0, scale=-0.5,
        )
        for q in range(SLOT_Q // QT):
            qt_i = s * (SLOT_Q // QT) + q
            ot = outp.tile([QT, D], _F32)
            nc.vector.tensor_scalar_mul(
                ot[:], muq_all[:, qt_i * D:(qt_i + 1) * D], invs[:, q:q + 1]
            )
            nc.sync.dma_start(aps["out"][qt_i * QT:(qt_i + 1) * QT, :], ot[:])


@with_exitstack
def _body_split(ctx: ExitStack, tc, aps, bias_val):
    """bf16 hi/lo strategy. aps: dict of DRAM APs."""
    nc = tc.nc
    PSUM = bass.MemorySpace.PSUM

    const = ctx.enter_context(tc.tile_pool(name="const", bufs=1))
    qdp = ctx.enter_context(tc.tile_pool(name="qdp", bufs=1))
    kdp = ctx.enter_context(tc.tile_pool(name="kdp", bufs=4))
    vnp = ctx.enter_context(tc.tile_pool(name="vnp", bufs=4))
    expp = ctx.enter_context(tc.tile_pool(name="expp", bufs=3))
    ehp = ctx.enter_context(tc.tile_pool(name="ehp", bufs=3))
    elp = ctx.enter_context(tc.tile_pool(name="elp", bufs=3))
    stp = ctx.enter_context(tc.tile_pool(name="stp", bufs=2, space=PSUM))
    mup = ctx.enter_context(tc.tile_pool(name="mup", bufs=2, space=PSUM))
    tpp = ctx.enter_context(tc.tile_pool(name="tpp", bufs=2, space=PSUM))
    sbp = ctx.enter_context(tc.tile_pool(name="sbp", bufs=1))
    smallp = ctx.enter_context(tc.tile_pool(name="smallp", bufs=4))
    outp = ctx.enter_context(tc.tile_pool(name="outp", bufs=3))

    ident = const.tile([64, 64], _F32)
    cmasks.make_identity(nc, ident[:])
    bias_t = const.tile([128, 1], _F32)
    nc.vector.memset(bias_t[:], float(bias_val))

    qdh_sb = qdp.tile([128, NQ_CORE], _BF16)
    nc.sync.dma_start(qdh_sb[:], aps["qdh"][:])
    qdl_sb = qdp.tile([64, NQ_CORE], _BF16)
    nc.sync.dma_start(qdl_sb[:], aps["qdl"][:])

    mu_sb = sbp.tile([64, NQ_CORE], _F32)
    muq_all = sbp.tile([128, NT_CORE * D], _F32)
    ln_all = sbp.tile([128, NT_CORE], _F32)

    step_base = 0
    for s in range(SLOTS):
        ext = SLOT_EXTENTS[s]
        q_lo = s * SLOT_Q
        mu_ps = mup.tile([QT, SLOT_Q], _F32)  # rows 0-63 hi, 64-127 lo
        for i in range(ext):
            st = step_base + i
            kt = kdp.tile([QT, QT], _BF16)
            nc.sync.dma_start(kt[:], aps["kd"][:, st * QT:(st + 1) * QT])
            vt = vnp.tile([QT, QT], _BF16)
            nc.sync.dma_start(vt[:], aps["vn"][st * QT:(st + 1) * QT, :])

            ps = stp.tile([QT, SLOT_Q], _F32)
            nc.tensor.matmul(
                ps[:], lhsT=kt[:], rhs=qdh_sb[:, q_lo:q_lo + SLOT_Q],
                start=True, stop=False,
            )
            nc.tensor.matmul(
                ps[:], lhsT=kt[0:64, :], rhs=qdl_sb[:, q_lo:q_lo + SLOT_Q],
                start=False, stop=True,
            )
            et = expp.tile([QT, SLOT_Q], _F32)
            nc.scalar.activation(
                et[:], ps[:], mybir.ActivationFunctionType.Exp,
                bias=bias_t[:], scale=1.0,
            )
            if i < 4:
                # step i's k-tile is the (3-i)'th q-tile block's diagonal:
                # keep element (k, q) iff q - k - 128*(3-i) >= 0
                nc.gpsimd.affine_select(
                    out=et[:], in_=et[:],
                    compare_op=mybir.AluOpType.is_ge,
                    fill=0.0,
                    base=-QT * (3 - i),
                    pattern=[[1, SLOT_Q]],
                    channel_multiplier=-1,
                )
            eth = ehp.tile([QT, SLOT_Q], _BF16)
            nc.vector.tensor_copy(eth[:], et[:])
            etl = elp.tile([QT, SLOT_Q], _BF16)
            nc.vector.tensor_sub(etl[:], et[:], eth[:])
            nc.tensor.matmul(
                mu_ps[:], lhsT=vt[:], rhs=eth[:],
                start=(i == 0), stop=False,
            )
            nc.tensor.matmul(
                mu_ps[:], lhsT=vt[:], rhs=etl[:],
                start=False, stop=(i == ext - 1),
            )
        step_base += ext

        # mu = hi half + lo half (one PSUM operand max per DVE op)
        lo_sb = smallp.tile([64, SLOT_Q], _F32, tag="losb")
        nc.scalar.copy(lo_sb[:], mu_ps[64:128, :])
        nc.vector.tensor_add(mu_sb[:, q_lo:q_lo + SLOT_Q], mu_ps[0:64, :], lo_sb[:])

        for q in range(SLOT_Q // QT):
            qt_i = s * (SLOT_Q // QT) + q
            tp = tpp.tile([QT, 64], _F32)
            nc.tensor.transpose(
                tp[:], mu_sb[:, qt_i * QT:(qt_i + 1) * QT], ident[:]
            )
            muq = muq_all[:, qt_i * D:(qt_i + 1) * D]
            nc.scalar.copy(muq, tp[:, :D])
            sq = smallp.tile([QT, D], _F32)
            nc.vector.tensor_mul(sq[:], muq, muq)
            red = smallp.tile([QT, 1], _F32)
            nc.vector.reduce_sum(red[:], sq[:], axis=mybir.AxisListType.X)
            # |l| = -l = 2*mu0^2 - sum(mu_d^2)  (l is always < 0 here)
            nc.vector.scalar_tensor_tensor(
                out=ln_all[:, qt_i:qt_i + 1],
                in0=sq[:, 0:1],
                scalar=2.0,
                in1=red[:],
                op0=mybir.AluOpType.mult,
                op1=mybir.AluOpType.subtract,
            )

    # grouped sqrt (single ACT table switch) + reciprocal + final scale
    sqv = sbp.tile([128, NT_CORE], _F32)
    nc.scalar.activation(
        sqv[:], ln_all[:], mybir.ActivationFunctionType.Sqrt,
        bias=0.0, scale=1.0,
    )
    inv = sbp.tile([128, NT_CORE], _F32)
    nc.vector.reciprocal(inv[:], sqv[:])
    for qt_i in range(NT_CORE):
        ot = outp.tile([QT, D], _F32)
        nc.vector.tensor_scalar_mul(
            ot[:], muq_all[:, qt_i * D:(qt_i + 1) * D], inv[:, qt_i:qt_i + 1]
        )
        nc.sync.dma_start(aps["out"][qt_i * QT:(qt_i + 1) * QT, :], ot[:])


@with_exitstack
def _body_f32(ctx: ExitStack, tc, aps, bias_val):
    """Exact-fp32 fallback strategy."""
    nc = tc.nc
    PSUM = bass.MemorySpace.PSUM

    const = ctx.enter_context(tc.tile_pool(name="const", bufs=1))
    qdp = ctx.enter_context(tc.tile_pool(name="qdp", bufs=1))
    kdp = ctx.enter_context(tc.tile_pool(name="kdp", bufs=4))
    vnp = ctx.enter_context(tc.tile_pool(name="vnp", bufs=4))
    expp = ctx.enter_context(tc.tile_pool(name="expp", bufs=3))
    stp = ctx.enter_context(tc.tile_pool(name="stp", bufs=2, space=PSUM))
    mup = ctx.enter_context(tc.tile_pool(name="mup", bufs=2, space=PSUM))
    tpp = ctx.enter_context(tc.tile_pool(name="tpp", bufs=2, space=PSUM))
    sbp = ctx.enter_context(tc.tile_pool(name="sbp", bufs=1))
    smallp = ctx.enter_context(tc.tile_pool(name="smallp", bufs=4))
    outp = ctx.enter_context(tc.tile_pool(name="outp", bufs=3))

    ident = const.tile([64, 64], _F32)
    cmasks.make_identity(nc, ident[:])
    bias_t = const.tile([128, 1], _F32)
    nc.vector.memset(bias_t[:], float(bias_val))

    qd_sb = qdp.tile([64, NQ_CORE], _F32)
    nc.sync.dma_start(qd_sb[:], aps["qd"][:])

    mu_sb = sbp.tile([64, NQ_CORE], _F32)
    muq_all = sbp.tile([128, NT_CORE * D], _F32)
    ln_all = sbp.tile([128, NT_CORE], _F32)

    step_base = 0
    for s in range(SLOTS):
        ext = SLOT_EXTENTS[s]
        q_lo = s * SLOT_Q
        mu_ps = mup.tile([64, SLOT_Q], _F32)
        for i in range(ext):
            st = step_base + i
            kt = kdp.tile([64, QT], _F32)
            nc.sync.dma_start(kt[:], aps["kd"][:, st * QT:(st + 1) * QT])
            vt = vnp.tile([QT, D], _F32)
            nc.sync.dma_start(vt[:], aps["vn"][st * QT:(st + 1) * QT, :])

            ps = stp.tile([QT, SLOT_Q], _F32)
            nc.tensor.matmul(
                ps[:], lhsT=kt[:], rhs=qd_sb[:, q_lo:q_lo + SLOT_Q],
                start=True, stop=True,
            )
            et = expp.tile([QT, SLOT_Q], _F32)
            nc.scalar.activation(
                et[:], ps[:], mybir.ActivationFunctionType.Exp,
                bias=bias_t[:], scale=1.0,
            )
            if i < 4:
                nc.gpsimd.affine_select(
                    out=et[:], in_=et[:],
                    compare_op=mybir.AluOpType.is_ge,
                    fill=0.0,
                    base=-QT * (3 - i),
                    pattern=[[1, SLOT_Q]],
                    channel_multiplier=-1,
                )
            nc.tensor.matmul(
                mu_ps[:], lhsT=vt[:], rhs=et[:],
                start=(i == 0), stop=(i == ext - 1),
            )
        step_base += ext

        nc.vector.tensor_copy(mu_sb[:, q_lo:q_lo + SLOT_Q], mu_ps[:])
        for q in range(SLOT_Q // QT):
            qt_i = s * (SLOT_Q // QT) + q
            tp = tpp.tile([QT, 64], _F32)
            nc.tensor.transpose(
                tp[:], mu_sb[:, qt_i * QT:(qt_i + 1) * QT], ident[:]
            )
            muq = muq_all[:, qt_i * D:(qt_i + 1) * D]
            nc.scalar.copy(muq, tp[:, :D])
            sq = smallp.tile([QT, D], _F32)
            nc.vector.tensor_mul(sq[:], muq, muq)
            red = smallp.tile([QT, 1], _F32)
            nc.vector.reduce_sum(red[:], sq[:], axis=mybir.AxisListType.X)
            nc.vector.scalar_tensor_tensor(
                out=ln_all[:, qt_i:qt_i + 1],
                in0=sq[:, 0:1],
                scalar=2.0,
                in1=red[:],
                op0=mybir.AluOpType.mult,
                op1=mybir.AluOpType.subtract,
            )

    sqv = sbp.tile([128, NT_CORE], _F32)
    nc.scalar.activation(
        sqv[:], ln_all[:], mybir.ActivationFunctionType.Sqrt,
        bias=0.0, scale=1.0,
    )
    inv = sbp.tile([128, NT_CORE], _F32)
    nc.vector.reciprocal(inv[:], sqv[:])
    for qt_i in range(NT_CORE):
        ot = outp.tile([QT, D], _F32)
        nc.vector.tensor_scalar_mul(
            ot[:], muq_all[:, qt_i * D:(qt_i + 1) * D], inv[:, qt_i:qt_i + 1]
        )
        nc.sync.dma_start(aps["out"][qt_i * QT:(qt_i + 1) * QT, :], ot[:])


def _build_program(bias_val):
    key = (round(float(bias_val), 12), _STRATEGY)
    if key in _cache:
        return _cache[key]
    nc = bacc.Bacc(
        "TRN2",
        target_bir_lowering=False,
        debug=False,
        enable_asserts=False,
    )
    aps = {}
    if _STRATEGY == "k66q":
        aps["qd66"] = nc.dram_tensor("qd66", [_KSTACK, NQ_CORE], _BF16, kind="ExternalInput").ap()
        aps["kd66"] = nc.dram_tensor("kd66", [_KSTACK, TOTAL_STEPS2 * QT], _BF16, kind="ExternalInput").ap()
        aps["vnt"] = nc.dram_tensor("vnt", [128, TOTAL_STEPS2 * QT], _FP16, kind="ExternalInput").ap()
        aps["out"] = nc.dram_tensor("out", [128, NT_CORE * D], _F32, kind="ExternalOutput").ap()
    elif _STRATEGY == "k66v2":
        aps["qd66"] = nc.dram_tensor("qd66", [_KSTACK, NQ_CORE], _BF16, kind="ExternalInput").ap()
        aps["kd66"] = nc.dram_tensor("kd66", [_KSTACK, TOTAL_STEPS * QT], _BF16, kind="ExternalInput").ap()
        aps["vnt"] = nc.dram_tensor("vnt", [128, TOTAL_STEPS * QT], _FP16, kind="ExternalInput").ap()
        # fp16 output: |out| <= ~5 so fp16 rounding adds ~5e-4 rel error,
        # and the tail output DMA halves
        aps["out"] = nc.dram_tensor("out", [128, NT_CORE * D], _FP16, kind="ExternalOutput").ap()
    elif _STRATEGY == "k66":
        aps["qd66"] = nc.dram_tensor("qd66", [_KSTACK, NQ_CORE], _BF16, kind="ExternalInput").ap()
        aps["kd66"] = nc.dram_tensor("kd66", [_KSTACK, TOTAL_STEPS * QT], _BF16, kind="ExternalInput").ap()
        aps["vn"] = nc.dram_tensor("vn", [TOTAL_STEPS * QT, 128], _FP16, kind="ExternalInput").ap()
    elif _STRATEGY in ("split", "fp16", "mixed"):
        kq_dt = _BF16 if _STRATEGY in ("split", "mixed") else _FP16
        pv_dt = _BF16 if _STRATEGY == "split" else _FP16
        aps["qdh"] = nc.dram_tensor("qdh", [128, NQ_CORE], kq_dt, kind="ExternalInput").ap()
        aps["qdl"] = nc.dram_tensor("qdl", [64, NQ_CORE], kq_dt, kind="ExternalInput").ap()
        aps["kd"] = nc.dram_tensor("kd", [128, TOTAL_STEPS * QT], kq_dt, kind="ExternalInput").ap()
        aps["vn"] = nc.dram_tensor("vn", [TOTAL_STEPS * QT, 128], pv_dt, kind="ExternalInput").ap()
    else:
        aps["qd"] = nc.dram_tensor("qd", [64, NQ_CORE], _F32, kind="ExternalInput").ap()
        aps["kd"] = nc.dram_tensor("kd", [64, TOTAL_STEPS * QT], _F32, kind="ExternalInput").ap()
        aps["vn"] = nc.dram_tensor("vn", [TOTAL_STEPS * QT, D], _F32, kind="ExternalInput").ap()
    if _STRATEGY not in ("k66v2", "k66q"):
        aps["out"] = nc.dram_tensor("out", [NQ_CORE, D], _F32, kind="ExternalOutput").ap()
    with tile.TileContext(nc) as tc:
        if _STRATEGY == "k66q":
            _body_k66q(tc, aps, bias_val)
        elif _STRATEGY == "k66v2":
            _body_k66v2(tc, aps, bias_val)
        elif _STRATEGY == "k66":
            _body_k66(tc, aps, bias_val)
        elif _STRATEGY == "mixed":
            _body_fp16(tc, aps, bias_val, kq_dt=_BF16)
        elif _STRATEGY == "fp16":
            _body_fp16(tc, aps, bias_val, kq_dt=_FP16)
        elif _STRATEGY == "split":
            _body_split(tc, aps, bias_val)
        else:
            _body_f32(tc, aps, bias_val)
    nc.compile()
    _cache[key] = nc
    return nc


def _hilo(x, np_dt):
    hi = x.astype(np_dt)
    lo = (x - hi.astype(np.float32)).astype(np_dt)
    return hi, lo


def _prep_core_inputs_q(Q, b, half, a_scale, poison):
    """k66q staging: 8 slots of 256 queries; slot s holds the half's group
    with the s-th smallest causal extent; k iterated DESCENDING from the
    diagonal; steps beyond the group's real extent get poison K columns."""
    groups = HALF_GROUPS2[half]
    Qb = Q[b]
    qd = np.empty((64, NQ_CORE), np.float32)
    kd = np.empty((64, TOTAL_STEPS2 * QT), np.float32)
    vn = np.zeros((TOTAL_STEPS2 * QT, D), np.float32)
    step_base = 0
    for s, m in enumerate(groups):
        ext = EXTS2[s]
        qd[:, s * SLOT_Q2:(s + 1) * SLOT_Q2] = (
            Qb[m * SLOT_Q2:(m + 1) * SLOT_Q2, :].T * a_scale
        )
        n_real = 2 * m + 2  # causal extent of this 2-qtile group in k-tiles
        for i in range(ext):
            c0 = (step_base + i) * QT
            if i < n_real:
                j = 2 * m + 1 - i  # descending from the diagonal
                blk = Qb[j * QT:(j + 1) * QT, :]
                kdb = blk.T.copy()
                kdb[0, :] = -kdb[0, :]
                kd[:, c0:c0 + QT] = kdb
                vn[c0:c0 + QT, :] = blk
            else:
                kd[:, c0:c0 + QT] = 0.0
                kd[0, c0:c0 + QT] = poison
        step_base += ext
    k0h, k0l = _hilo(kd[0:1], _BF16_NP)
    q0h, q0l = _hilo(qd[0:1], _BF16_NP)
    kd66 = np.empty((_KSTACK, TOTAL_STEPS2 * QT), _BF16_NP)
    kd66[0] = k0h
    kd66[1] = k0l
    kd66[2] = k0h
    kd66[3:] = kd[1:].astype(_BF16_NP)
    qd66 = np.empty((_KSTACK, NQ_CORE), _BF16_NP)
    qd66[0] = q0h
    qd66[1] = q0h
    qd66[2] = q0l
    qd66[3:] = qd[1:].astype(_BF16_NP)
    vh, vl = _hilo(vn, np.float16)
    vns = np.concatenate([vh, vl], axis=1)
    vnt = vns.reshape(TOTAL_STEPS2, QT, 128).transpose(1, 0, 2)
    vnt = np.ascontiguousarray(vnt.reshape(QT, TOTAL_STEPS2 * 128))
    return {"qd66": qd66, "kd66": kd66, "vnt": vnt}


def _prep_core_inputs(Q, b, half, a_scale, poison):
    """Build per-core input arrays. a_scale folded into q."""
    groups = HALF_GROUPS[half]
    exts = SLOT_EXTENTS
    n_steps = sum(exts)
    Qb = Q[b]  # [L, D]
    qd = np.empty((64, NQ_CORE), np.float32)
    kd = np.empty((64, n_steps * QT), np.float32)
    vn = np.zeros((n_steps * QT, D), np.float32)
    step_base = 0
    for s, g in enumerate(groups):
        ext = exts[s]
        qd[:, s * SLOT_Q:(s + 1) * SLOT_Q] = (
            Qb[g * SLOT_Q:(g + 1) * SLOT_Q, :].T * a_scale
        )
        n_real = 4 * g + 4  # causal extent of this group in k-tiles
        for i in range(ext):
            c0 = (step_base + i) * QT
            if i < n_real:
                j = 4 * g + 3 - i  # descending from the diagonal
                blk = Qb[j * QT:(j + 1) * QT, :]  # [128, 64]
                kdb = blk.T.copy()
                kdb[0, :] = -kdb[0, :]  # Lorentz signature on time component
                kd[:, c0:c0 + QT] = kdb
                vn[c0:c0 + QT, :] = blk
            else:
                kd[:, c0:c0 + QT] = 0.0
                kd[0, c0:c0 + QT] = poison
                # vn stays zero
        step_base += ext
    if _STRATEGY in ("k66", "k66v2"):
        # kd rows already carry the Lorentz sign on row 0 (time).
        k0h, k0l = _hilo(kd[0:1], _BF16_NP)      # signed time component
        q0h, q0l = _hilo(qd[0:1], _BF16_NP)
        kd66 = np.empty((_KSTACK, n_steps * QT), _BF16_NP)
        kd66[0] = k0h
        kd66[1] = k0l
        kd66[2] = k0h
        kd66[3:] = kd[1:].astype(_BF16_NP)
        qd66 = np.empty((_KSTACK, NQ_CORE), _BF16_NP)
        qd66[0] = q0h
        qd66[1] = q0h
        qd66[2] = q0l
        qd66[3:] = qd[1:].astype(_BF16_NP)
        vh, vl = _hilo(vn, np.float16)
        vns = np.concatenate([vh, vl], axis=1)   # [steps*128, 128]
        if _STRATEGY == "k66v2":
            # pre-transposed so the kernel's [128 keys, steps*128] SBUF
            # layout loads with fully contiguous per-partition DMA lines
            vnt = vns.reshape(n_steps, QT, 128).transpose(1, 0, 2)
            vnt = np.ascontiguousarray(vnt.reshape(QT, n_steps * 128))
            return {"qd66": qd66, "kd66": kd66, "vnt": vnt}
        return {"qd66": qd66, "kd66": kd66, "vn": np.ascontiguousarray(vns)}
    if _STRATEGY not in ("split", "fp16", "mixed"):
        return {"qd": qd, "kd": kd, "vn": vn}
    np_dt = _BF16_NP if _STRATEGY in ("split", "mixed") else np.float16
    pv_np = _BF16_NP if _STRATEGY == "split" else np.float16
    qh, ql = _hilo(qd, np_dt)
    kh, kl = _hilo(kd, np_dt)
    vh, vl = _hilo(vn, pv_np)
    qdh = np.empty((128, NQ_CORE), np_dt)
    qdh[0:64] = qh
    qdh[64:128] = qh  # replicated: both halves of the K-stack see Q_hi
    kds = np.concatenate([kh, kl], axis=0)       # [128, steps*128]
    vns = np.concatenate([vh, vl], axis=1)       # [steps*128, 128]
    return {"qdh": qdh, "qdl": ql, "kd": np.ascontiguousarray(kds),
            "vn": np.ascontiguousarray(vns)}


def _mask_fixup(out, Q, mask, scale_v, bias_v):
    """Reference masks by QUERY row: a masked row becomes a uniform softmax
    over ALL L keys (causal entries equally -inf). Recompute those rows."""
    for b in range(B):
        rows = np.nonzero(mask[b])[0]
        if len(rows) == 0:
            continue
        mu = Q[b].mean(axis=0)  # uniform attention over all keys
        l_norm = -mu[0] ** 2 + np.sum(mu[1:] ** 2)
        denom = np.sqrt(max(abs(l_norm), EPS))
        out[b, rows, :] = (mu / denom)[None, :]
    return out


LAST_EXEC_NS = None
LAST_RESULTS = None


def kernel(Q, mask, scale, bias, _trace=False):
    global LAST_EXEC_NS, LAST_RESULTS
    Q = np.ascontiguousarray(np.asarray(Q, dtype=np.float32))
    mask_np = np.asarray(mask).astype(bool).reshape(B, L)
    scale_v = float(np.asarray(scale).reshape(-1)[0])
    bias_v = float(np.asarray(bias).reshape(-1)[0]) if np.asarray(bias).size else float(bias)

    a_scale = 2.0 / scale_v              # folded into q host-side
    b0 = 2.0 / scale_v + bias_v          # activation bias immediate
    poison = -(500.0 + abs(b0)) / a_scale

    if _trace:
        _ensure_ntff_hook()
    nc = _build_program(b0)

    in_maps = []
    for c in range(N_CORES):
        b, half = c // 2, c % 2
        if _STRATEGY == "k66q":
            in_maps.append(_prep_core_inputs_q(Q, b, half, a_scale, poison))
        else:
            in_maps.append(_prep_core_inputs(Q, b, half, a_scale, poison))

    res = bass_utils.run_bass_kernel_spmd(
        nc, in_maps, core_ids=list(range(N_CORES)), trace=_trace
    )
    LAST_EXEC_NS = res.exec_time_ns
    LAST_RESULTS = res

    out = np.empty((B, L, D), np.float32)
    for c in range(N_CORES):
        o = res.results[c]["out"]
        if _STRATEGY in ("k66v2", "k66q"):
            # [128, 16*64] -> [2048, 64] (q-tile-major columns)
            o = np.ascontiguousarray(
                o.reshape(QT, NT_CORE, D).transpose(1, 0, 2).reshape(NQ_CORE, D)
            )
        b, half = c // 2, c % 2
        if _STRATEGY == "k66q":
            for s, m in enumerate(HALF_GROUPS2[half]):
                out[b, m * SLOT_Q2:(m + 1) * SLOT_Q2, :] = \
                    o[s * SLOT_Q2:(s + 1) * SLOT_Q2, :]
        else:
            for s, g in enumerate(HALF_GROUPS[half]):
                out[b, g * SLOT_Q:(g + 1) * SLOT_Q, :] = o[s * SLOT_Q:(s + 1) * SLOT_Q, :]

    if mask_np.any():
        out = _mask_fixup(out, Q, mask_np, scale_v, bias_v)
    return out

